# revision 1
# baseline (speedup 1.0000x reference)
"""Single-head causal attention (B=8, T=4096, EMB=1024, HEAD=64) on 8 trn2 cores.

Strategy: data-parallel over batch, one batch element per NeuronCore.

Per core (all matmuls in bf16, fp32 PSUM accumulation):
  1. QKV projection from host-pretransposed xT [1024, 4096]:
       KQ^T [128, 4096]  (rows 0:64 = K^T, 64:128 = Q^T), via W[:, 0:128] stationary
       V    [4096, 64]   natural layout, via xT-chunk stationary x Wv moving
  2. Q^T copied to partitions 0:63 (sbuf->sbuf DMA) so scores matmuls can
     contract over d=64 on partitions 0:63.
  3. Scores S^T[s, t] = K @ Q^T per (s-chunk 128, t-tile 512), PSUM fp32.
     No max-subtraction: scores ~ N(0, 0.41), exp is safe in fp32.
  4. exp via ScalarE directly from PSUM (scale=1/8 folded in), bf16 out.
     Causal: only tiles with t >= s computed; diagonal 128x128 block masked
     by a 0/1 multiply after exp.
  5. PV: P^T tile [128s, 128t] stationary, V-with-ones-column [128, 65] moving
     -> accumulates [O_unnorm | rowsum] in natural [t, 65] layout.
  6. out = O_unnorm * reciprocal(rowsum) (per-partition scalar), DMA out fp32.
"""

from contextlib import ExitStack

import numpy as np
import ml_dtypes

B, T, EMB, HEAD = 8, 4096, 1024, 64
KCH = EMB // 128          # 8 contraction chunks
NTT = T // 512            # 8 t-tiles of 512
NTS = T // 128            # 32 t-subtiles / s-chunks of 128
BF16 = ml_dtypes.bfloat16

_CACHE = {}


def _build_program():
    import concourse.bacc as bacc
    import concourse.tile as tile
    from concourse import mybir
    from concourse.masks import make_identity

    fp32 = mybir.dt.float32
    bf16 = mybir.dt.bfloat16
    EXP = mybir.ActivationFunctionType.Exp

    nc = bacc.Bacc("TRN2", target_bir_lowering=False, debug=False)
    xt_ap = nc.dram_tensor("xt", [EMB, T], bf16, kind="ExternalInput").ap()
    w_ap = nc.dram_tensor("w", [EMB, 192], bf16, kind="ExternalInput").ap()
    mask_ap = nc.dram_tensor("mask", [128, 128], bf16, kind="ExternalInput").ap()
    o_ap = nc.dram_tensor("o", [T, HEAD], fp32, kind="ExternalOutput").ap()

    with tile.TileContext(nc) as tc:
        with (
            tc.tile_pool(name="consts", bufs=1) as consts,
            tc.tile_pool(name="outs", bufs=4) as outs,
        ):
            # ---------- constants ----------
            w_sb = consts.tile([128, KCH, 192], bf16, tag="w")
            for k in range(KCH):
                nc.sync.dma_start(out=w_sb[:, k, :], in_=w_ap[k * 128:(k + 1) * 128, :])
            mask_sb = consts.tile([128, 128], bf16, tag="mask")
            nc.sync.dma_start(out=mask_sb, in_=mask_ap)
            ident_sb = consts.tile([128, 128], fp32, tag="ident")
            make_identity(nc, ident_sb)
            # V with ones column: [128, 65] per s-chunk; col 64 preset to 1.0
            vt_sb = consts.tile([128, NTS * 65], bf16, tag="vt")
            nc.gpsimd.memset(vt_sb, 1.0)

            kq_sb = consts.tile([128, T], bf16, tag="kq")
            qk_sb = consts.tile([128, T], bf16, tag="qk")

            # ---------- phase 1: load x, project (k-outer: PE starts as
            # each x chunk lands; KQ in two half-T passes + V share 8 banks)
            with (
                tc.tile_pool(name="xp", bufs=1) as xp,
                tc.tile_pool(name="ps_kq", bufs=1, space="PSUM") as ps_kq,
                tc.tile_pool(name="ps_v", bufs=1, space="PSUM") as ps_v,
            ):
                xt_sb = xp.tile([128, KCH, T], bf16, tag="xt")
                for k in range(KCH):
                    nc.sync.dma_start(
                        out=xt_sb[:, k, :], in_=xt_ap[k * 128:(k + 1) * 128, :]
                    )

                pkq = []
                for j in range(4):
                    pkq_j = ps_kq.tile([128, 512], fp32, tag=f"kq{j}")
                    pkq.append(pkq_j)
                pv = []
                for g in range(4):
                    pv_g = ps_v.tile([128, 512], fp32, tag=f"v{g}")
                    pv.append(pv_g)

                # pass 1 over k: KQ t-tiles 0..3 and all V accumulators
                for k in range(KCH):
                    for j in range(4):
                        nc.tensor.matmul(
                            pkq[j],
                            w_sb[:, k, 0:128],
                            xt_sb[:, k, j * 512:(j + 1) * 512],
                            start=(k == 0),
                            stop=(k == KCH - 1),
                            skip_group_check=True,
                        )
                    for i in range(NTS):
                        # start=True clears the WHOLE bank's has_written bits,
                        # so only the first accumulator in each bank may issue
                        # it; the rest overwrite-on-first-write via per-element
                        # has_written.
                        nc.tensor.matmul(
                            pv[i // 8][:, (i % 8) * 64:(i % 8 + 1) * 64],
                            xt_sb[:, k, i * 128:(i + 1) * 128],
                            w_sb[:, k, 128:192],
                            start=(k == 0 and i % 8 == 0),
                            stop=(k == KCH - 1),
                            skip_group_check=True,
                        )
                for j in range(4):
                    nc.vector.tensor_copy(kq_sb[:, j * 512:(j + 1) * 512], pkq[j])
                for i in range(NTS):
                    nc.vector.tensor_copy(
                        vt_sb[:, i * 65:i * 65 + 64],
                        pv[i // 8][:, (i % 8) * 64:(i % 8 + 1) * 64],
                    )
                # pass 2 over k (x fully resident): KQ t-tiles 4..7
                pkq2 = []
                for j in range(4):
                    pkq2_j = ps_kq.tile([128, 512], fp32, tag=f"kq{j}")
                    pkq2.append(pkq2_j)
                for k in range(KCH):
                    for j in range(4):
                        nc.tensor.matmul(
                            pkq2[j],
                            w_sb[:, k, 0:128],
                            xt_sb[:, k, (j + 4) * 512:(j + 5) * 512],
                            start=(k == 0),
                            stop=(k == KCH - 1),
                            skip_group_check=True,
                        )
                for j in range(4):
                    nc.vector.tensor_copy(
                        kq_sb[:, (j + 4) * 512:(j + 5) * 512], pkq2[j]
                    )
                # Q^T to low partitions for scores moving operand
                nc.sync.dma_start(out=qk_sb[0:64, :], in_=kq_sb[64:128, :])

            # ---------- phase 2: attention ----------
            phase2 = ExitStack()
            ptp = phase2.enter_context(tc.tile_pool(name="pt", bufs=1))
            ps_s = phase2.enter_context(tc.tile_pool(name="ps_s", bufs=2, space="PSUM"))
            ps_o = phase2.enter_context(tc.tile_pool(name="ps_o", bufs=1, space="PSUM"))
            pt = []
            for a in range(NTS):
                pt_a = ptp.tile([128, T - 128 * a], bf16, tag=f"pt{a}")
                pt.append(pt_a)

            def score_groups(a):
                """[(jstart, gsize), ...] groups of <=3 t-tiles for s-chunk a."""
                j0 = a // 4
                groups = []
                j = j0
                while j < NTT:
                    g = min(3, NTT - j)
                    groups.append((j, g))
                    j += g
                return groups

            def emit_scores(a):
                tiles = []
                for (jstart, g) in score_groups(a):
                    psg = ps_s.tile([128, 512 * g], fp32, tag="sg")
                    for idx in range(g):
                        j = jstart + idx
                        nc.tensor.matmul(
                            psg[:, idx * 512:(idx + 1) * 512],
                            kq_sb[0:64, a * 128:(a + 1) * 128],
                            qk_sb[0:64, j * 512:(j + 1) * 512],
                            start=True,
                            stop=True,
                        )
                    tiles.append((jstart, g, psg))
                return tiles

            def emit_exp(a, tiles):
                for (jstart, g, psg) in tiles:
                    skip = max(0, 128 * a - 512 * jstart)
                    out_lo = 512 * jstart + skip - 128 * a
                    out_hi = 512 * (jstart + g) - 128 * a
                    nc.scalar.activation(
                        pt[a][:, out_lo:out_hi],
                        psg[:, skip:512 * g],
                        EXP,
                        scale=0.125,
                    )
                # mask the diagonal 128x128 block (zero where s > t)
                nc.vector.tensor_mul(pt[a][:, 0:128], pt[a][:, 0:128], mask_sb)

            def emit_pv(i):
                po = ps_o.tile([128, 65], fp32, tag="o")
                for aa in range(i + 1):
                    nc.tensor.matmul(
                        po,
                        pt[aa][:, 128 * (i - aa):128 * (i - aa) + 128],
                        vt_sb[:, aa * 65:(aa + 1) * 65],
                        start=(aa == 0),
                        stop=(aa == i),
                    )
                dr = outs.tile([128, 1], fp32, tag="dr")
                nc.vector.reciprocal(dr, po[:, 64:65])
                o_sb = outs.tile([128, 64], fp32, tag="o_sb")
                nc.vector.tensor_scalar_mul(o_sb, po[:, 0:64], dr)
                nc.sync.dma_start(out=o_ap[i * 128:(i + 1) * 128, :], in_=o_sb)

            # software-pipelined: while ACT(a) drains, PE runs S(a+1); PV for
            # t-tile j fires once its last needed chunk (4j+3) is exp'd.
            tiles = emit_scores(0)
            for a in range(NTS):
                emit_exp(a, tiles)
                if a + 1 < NTS:
                    tiles = emit_scores(a + 1)
                if a >= 1:
                    emit_pv(a - 1)
            emit_pv(NTS - 1)
            phase2.close()

    nc.compile()
    return nc


def _get_nc():
    if "nc" not in _CACHE:
        _CACHE["nc"] = _build_program()
    return _CACHE["nc"]


def kernel(x, W):
    from concourse.bass_utils import run_bass_kernel_spmd

    x = np.asarray(x, dtype=np.float32)
    W = np.asarray(W, dtype=np.float32)
    assert x.shape == (B, T, EMB) and W.shape == (EMB, 3 * HEAD)

    xt = np.ascontiguousarray(x.transpose(0, 2, 1)).astype(BF16)  # [B, EMB, T]
    w16 = W.astype(BF16)
    mask = np.triu(np.ones((128, 128), np.float32)).astype(BF16)

    nc = _get_nc()
    in_maps = [{"xt": xt[b], "w": w16, "mask": mask} for b in range(B)]
    res = run_bass_kernel_spmd(nc, in_maps, list(range(B)))
    return np.stack([res.results[b]["o"] for b in range(B)]).astype(np.float32)



# revision 7
# speedup vs baseline: 1.0941x; 1.0941x over previous
"""Single-head causal attention (B=8, T=4096, EMB=1024, HEAD=64) on 8 trn2 cores.

Strategy: data-parallel over batch, one batch element per NeuronCore.

The per-core kernel is scalar-exp-bound (softmax exp runs only on the Scalar
engine at 1 col/cycle: causal T^2/2 elements = 67584 cols ~ 56us @1.2GHz), so
the whole kernel is organized to start exp as early as possible (~3.5us) and
keep the Scalar engine 100% fed:

  - x is host-packed as [128, KCH=8, T] and DMA'd per t-tile j (512 cols,
    1MB, ~2.9us each), so KQ^T for tile 0 is ready ~3.2us in.
  - Per tile j: KQ^T[128, 512] (8 k-chunk matmuls into 1 PSUM bank), then
    V[4 subtiles, 64] (32 matmuls into 1 bank), CAST to SBUF bf16.
  - Scores for s-chunks a<=4j+3 vs t-tile j stream into [128,<=1536] PSUM
    units (3 banks x2 buffered); ScalarE exp's each unit (scale=1/8 folded),
    bf16 out into a per-tile P^T buffer (exact causal widths - no wasted exp
    columns). Diagonal 128x128 blocks masked by 0/1 multiply after exp.
  - PV: per t-subtile i, chain of i+1 matmuls (stationary P^T slice
    [128,128], moving V-with-ones [128,65]) accumulates [O|rowsum]; PSUM
    bank shared with the V accumulator (tag rotation orders V(j) and the
    chains of tile j-1 on one bank). out = O * reciprocal(rowsum).
  - Emission is software-pipelined so the PE never blocks the scalar engine:
    next tile's KQ/V and the previous tile's PV chains are issued between
    score units of the current tile.

PSUM budget (8 banks): scores 2x[128,1536] (6) + KQ [128,512] (1) + V/PV
shared (1).
"""

import numpy as np
import ml_dtypes

B, T, EMB, HEAD = 8, 4096, 1024, 64
KCH = EMB // 128          # 8 contraction chunks
NTT = T // 512            # 8 t-tiles of 512
NTS = T // 128            # 32 t-subtiles / s-chunks of 128
UNIT = 1536               # max score-unit width (3 PSUM banks)
BF16 = ml_dtypes.bfloat16

_CACHE = {}


def _tile_slots(j):
    """[(a, t0, w, pt_off), ...] score slots for t-tile j.

    All slots are 512 wide (PSUM-bank aligned: every score matmul may then
    issue start=True without wiping a neighbour slot's bank). For diagonal
    chunks (a > 4j) the leading 128*(a-4j) columns hold t<s garbage that no
    PV chain ever reads (chain i reads column 128i with i >= a)."""
    return [(a, 512 * j, 512, 512 * a) for a in range(4 * j + 4)]


def _units(slots):
    """Greedy-pack slots into units of total width <= UNIT."""
    units = []
    cur, cw = [], 0
    for s in slots:
        if cur and cw + s[2] > UNIT:
            units.append(cur)
            cur, cw = [], 0
        cur.append(s)
        cw += s[2]
    if cur:
        units.append(cur)
    return units


def _build_program():
    import concourse.bacc as bacc
    import concourse.tile as tile
    from concourse import mybir

    fp32 = mybir.dt.float32
    bf16 = mybir.dt.bfloat16
    EXP = mybir.ActivationFunctionType.Exp

    PTW = 512 * NTS  # 16384: per-tile P^T buffer, slot a at column 512*a

    nc = bacc.Bacc("TRN2", target_bir_lowering=False, debug=False)
    xt_ap = nc.dram_tensor("xt", [128, KCH, T], bf16, kind="ExternalInput").ap()
    w_ap = nc.dram_tensor("w", [128, KCH, 192], bf16, kind="ExternalInput").ap()
    mask_ap = nc.dram_tensor("mask", [128, 128], bf16, kind="ExternalInput").ap()
    o_ap = nc.dram_tensor("o", [128, NTS, HEAD], fp32, kind="ExternalOutput").ap()

    with tile.TileContext(nc) as tc:
        with (
            tc.tile_pool(name="consts", bufs=1) as consts,
            tc.tile_pool(name="xp", bufs=3) as xp,
            tc.tile_pool(name="ptp", bufs=3) as ptp,
            tc.tile_pool(name="outs", bufs=4) as outs,
            tc.tile_pool(name="ps_s", bufs=2, space="PSUM") as ps_s,
            tc.tile_pool(name="ps_kq", bufs=1, space="PSUM") as ps_kq,
            tc.tile_pool(name="ps_vpv", bufs=1, space="PSUM") as ps_vpv,
        ):
            # ---------- constants ----------
            w_sb = consts.tile([128, KCH, 192], bf16, tag="w")
            nc.sync.dma_start(out=w_sb, in_=w_ap)
            mask_sb = consts.tile([128, 128], bf16, tag="mask")
            nc.sync.dma_start(out=mask_sb, in_=mask_ap)
            # V with ones column: slot aa is [128, 65], col 64 preset to 1.0
            vt_sb = consts.tile([128, NTS, 65], bf16, tag="vt")
            nc.gpsimd.memset(vt_sb, 1.0)
            kq_sb = consts.tile([128, T], bf16, tag="kq")
            qk_sb = consts.tile([64, T], bf16, tag="qk")
            # warm the exp table so ACT_TABLE_LOAD doesn't hit the first unit
            warm = consts.tile([128, 1], bf16, tag="warm")
            nc.gpsimd.memset(warm, 0.0)
            nc.scalar.activation(warm, warm, EXP, scale=1.0)

            xt_t = {}

            def dma_x(j):
                xt_t[j] = xp.tile([128, KCH, 512], bf16, tag="x", name=f"xt{j}")
                nc.sync.dma_start(out=xt_t[j], in_=xt_ap[:, :, j * 512:(j + 1) * 512])

            def emit_kqv(j):
                """KQ^T tile j + V tile j; cast/copy to SBUF; Q-half to qk_sb."""
                kq_ps = ps_kq.tile([128, 512], fp32, tag="kq")
                for k in range(KCH):
                    nc.tensor.matmul(
                        kq_ps,
                        w_sb[:, k, 0:128],
                        xt_t[j][:, k, :],
                        start=(k == 0),
                        stop=(k == KCH - 1),
                    )
                nc.vector.tensor_copy(kq_sb[:, j * 512:(j + 1) * 512], kq_ps)
                nc.sync.dma_start(
                    out=qk_sb[:, j * 512:(j + 1) * 512],
                    in_=kq_sb[64:128, j * 512:(j + 1) * 512],
                )
                v_ps = ps_vpv.tile([128, 4, 64], fp32, tag="vpv")
                for k in range(KCH):
                    for q in range(4):
                        nc.tensor.matmul(
                            v_ps[:, q, :],
                            xt_t[j][:, k, q * 128:(q + 1) * 128],
                            w_sb[:, k, 128:192],
                            start=(k == 0 and q == 0),
                            stop=(k == KCH - 1),
                            skip_group_check=True,
                        )
                nc.vector.tensor_copy(vt_sb[:, 4 * j:4 * j + 4, 0:64], v_ps)

            pt = {}

            def emit_unit(j, unit):
                uw = sum(s[2] for s in unit)
                base = unit[0][3]
                psu = ps_s.tile([128, UNIT], fp32, tag="s")
                for (a, t0, w, off) in unit:
                    nc.tensor.matmul(
                        psu[:, off - base:off - base + w],
                        kq_sb[0:64, a * 128:(a + 1) * 128],
                        qk_sb[:, t0:t0 + w],
                        start=True,
                        stop=True,
                    )
                nc.scalar.activation(
                    pt[j][:, base:base + uw], psu[:, 0:uw], EXP, scale=0.125
                )

            def emit_masks(j, slots):
                for (a, t0, w, off) in slots[-4:]:
                    d = off + 128 * (a - 4 * j)  # diagonal 128-block column
                    nc.vector.tensor_mul(
                        pt[j][:, d:d + 128], pt[j][:, d:d + 128], mask_sb
                    )

            def emit_chain(i, pool):
                jj = i // 4
                slots = _tile_slots(jj)
                po = pool.tile([128, 65], fp32, tag="vpv" if pool is ps_vpv else "kq", name=f"po{i}")
                for aa in range(i + 1):
                    (_, t0, _, off) = slots[aa]
                    col = off + 128 * i - t0
                    nc.tensor.matmul(
                        po,
                        pt[jj][:, col:col + 128],
                        vt_sb[:, aa, :],
                        start=(aa == 0),
                        stop=(aa == i),
                    )
                dr = outs.tile([128, 1], fp32, tag="dr")
                nc.vector.reciprocal(dr, po[:, 64:65])
                nc.vector.tensor_scalar_mul(og[:, i % 4, :], po[:, 0:64], dr)

            # ---------- pipeline ----------
            for j in range(3):
                dma_x(j)
            emit_kqv(0)

            og = None
            for j in range(NTT):
                slots = _tile_slots(j)
                units = _units(slots)
                pt[j] = ptp.tile([128, PTW], bf16, tag="pt", name=f"pt{j}")

                for u in units[0:2]:
                    emit_unit(j, u)
                if j >= 1:
                    og = outs.tile([128, 4, 64], fp32, tag="og")
                    for i in range(4 * (j - 1), 4 * j):
                        emit_chain(i, ps_vpv)
                    nc.sync.dma_start(
                        out=o_ap[:, 4 * (j - 1):4 * j, :], in_=og
                    )
                for u in units[2:3]:
                    emit_unit(j, u)
                if j + 3 < NTT:
                    dma_x(j + 3)
                if j + 1 < NTT:
                    emit_kqv(j + 1)
                for u in units[3:]:
                    emit_unit(j, u)
                emit_masks(j, slots)

            # tail: chains of tile 7, two banks in parallel (vpv + kq pools)
            og = outs.tile([128, 4, 64], fp32, tag="og")
            for n, i in enumerate(range(4 * (NTT - 1), 4 * NTT)):
                emit_chain(i, ps_vpv if n % 2 == 0 else ps_kq)
            nc.sync.dma_start(out=o_ap[:, 4 * (NTT - 1):4 * NTT, :], in_=og)

    nc.compile()
    return nc


def _get_nc():
    if "nc" not in _CACHE:
        _CACHE["nc"] = _build_program()
    return _CACHE["nc"]


def _prep_inputs(x, W):
    """Host-side packing shared by kernel() and test harnesses."""
    x = np.asarray(x, dtype=np.float32)
    W = np.asarray(W, dtype=np.float32)
    assert x.shape == (B, T, EMB) and W.shape == (EMB, 3 * HEAD)
    # [B, 128, KCH, T]: partition p of chunk k holds x[b, :, 128k+p]
    xt = np.ascontiguousarray(
        x.transpose(0, 2, 1).reshape(B, KCH, 128, T).transpose(0, 2, 1, 3)
    ).astype(BF16)
    w_r = np.ascontiguousarray(
        W.reshape(KCH, 128, 3 * HEAD)
    ).transpose(1, 0, 2).astype(BF16)
    w_r = np.ascontiguousarray(w_r)
    mask = np.triu(np.ones((128, 128), np.float32)).astype(BF16)
    return xt, w_r, mask


def kernel(x, W):
    from concourse.bass_utils import run_bass_kernel_spmd

    xt, w_r, mask = _prep_inputs(x, W)
    nc = _get_nc()
    in_maps = [{"xt": xt[b], "w": w_r, "mask": mask} for b in range(B)]
    res = run_bass_kernel_spmd(nc, in_maps, list(range(B)))
    # o[p, i, c] = out[128*i + p, c]
    return np.stack(
        [
            res.results[b]["o"].transpose(1, 0, 2).reshape(T, HEAD)
            for b in range(B)
        ]
    ).astype(np.float32)


# revision 13
# speedup vs baseline: 1.2783x; 1.1684x over previous
"""Single-head causal attention (B=8, T=4096, EMB=1024, HEAD=64) on 8 trn2 cores.

Strategy: data-parallel over batch, one batch element per NeuronCore.

The per-core kernel is scalar-exp-bound (softmax exp runs only on the Scalar
engine at 1 col/cycle: causal T^2/2 elements = 67584 cols ~ 56us @1.2GHz), so
the whole kernel is organized to start exp as early as possible (~3.5us) and
keep the Scalar engine 100% fed:

  - x is host-packed as [128, KCH=8, T] and DMA'd per t-tile j (512 cols,
    1MB, ~2.9us each), so KQ^T for tile 0 is ready ~3.2us in.
  - Per tile j: KQ^T[128, 512] (8 k-chunk matmuls into 1 PSUM bank), then
    V[4 subtiles, 64] (32 matmuls into 1 bank), CAST to SBUF bf16.
  - Scores for s-chunks a<=4j+3 vs t-tile j stream into [128,<=1536] PSUM
    units (3 banks x2 buffered); ScalarE exp's each unit (scale=1/8 folded),
    bf16 out into a per-tile P^T buffer (exact causal widths - no wasted exp
    columns). Diagonal 128x128 blocks masked by 0/1 multiply after exp.
  - PV: per t-subtile i, chain of i+1 matmuls (stationary P^T slice
    [128,128], moving V-with-ones [128,65]) accumulates [O|rowsum]; PSUM
    bank shared with the V accumulator (tag rotation orders V(j) and the
    chains of tile j-1 on one bank). out = O * reciprocal(rowsum).
  - Emission is software-pipelined so the PE never blocks the scalar engine:
    next tile's KQ/V and the previous tile's PV chains are issued between
    score units of the current tile.

PSUM budget (8 banks): scores 2x[128,1536] (6) + KQ [128,512] (1) + V/PV
shared (1).
"""

import numpy as np
import ml_dtypes

B, T, EMB, HEAD = 8, 4096, 1024, 64
KCH = EMB // 128          # 8 contraction chunks
NTT = T // 512            # 8 t-tiles of 512
NTS = T // 128            # 32 t-subtiles / s-chunks of 128
UNIT = 1536               # max score-unit width (3 PSUM banks)
FILL = 4                  # dummy LDWEIGHTS after each score unit (p-state)
BF16 = ml_dtypes.bfloat16

_CACHE = {}


def _tile_slots(j):
    """[(a, t0, w, pt_off, start_flag), ...] score slots for t-tile j.

    Exact causal widths, packed so every slot either begins at a PSUM bank
    boundary (start=True allowed: clearing the bank wipes nothing live) or
    is the 128-wide slot sharing the bank opened by the 384 one
    (start=False: its elements' has_written bits were cleared by that
    start). Order: 4j+1 full 512 slots, then widths 384,128 (one bank),
    then 256 (own bank, tail unused). pt layout mirrors psum: contiguous."""
    slots = []
    off = 0
    for a in range(4 * j + 1):
        slots.append((a, 512 * j, 512, off, True))
        off += 512
    for a, flag in ((4 * j + 1, True), (4 * j + 3, False), (4 * j + 2, True)):
        w = 512 * (j + 1) - 128 * a
        slots.append((a, 128 * a, w, off, flag))
        off += w
    return slots


def _units(slots):
    """Greedy-pack slots into units of <= UNIT psum columns, whole banks."""
    units = []
    cur, banks = [], 0
    for s in slots:
        nb = 1 if s[4] else 0  # the start=False 128 slot shares its bank
        if cur and (banks + nb) > UNIT // 512:
            units.append(cur)
            cur, banks = [], 0
        cur.append(s)
        banks += nb
    if cur:
        units.append(cur)
    return units


def _build_program():
    import concourse.bacc as bacc
    import concourse.tile as tile
    from concourse import mybir

    fp32 = mybir.dt.float32
    bf16 = mybir.dt.bfloat16
    EXP = mybir.ActivationFunctionType.Exp

    PTW = 512 * NTS  # 16384: per-tile P^T buffer, slot a at column 512*a

    nc = bacc.Bacc("TRN2", target_bir_lowering=False, debug=False)
    xt_ap = nc.dram_tensor("xt", [128, KCH, T], bf16, kind="ExternalInput").ap()
    w_ap = nc.dram_tensor("w", [128, KCH, 192], bf16, kind="ExternalInput").ap()
    mask_ap = nc.dram_tensor("mask", [128, 128], bf16, kind="ExternalInput").ap()
    o_ap = nc.dram_tensor("o", [128, NTS, HEAD], fp32, kind="ExternalOutput").ap()

    with tile.TileContext(nc) as tc:
        with (
            tc.tile_pool(name="consts", bufs=1) as consts,
            tc.tile_pool(name="xp", bufs=3) as xp,
            tc.tile_pool(name="ptp", bufs=3) as ptp,
            tc.tile_pool(name="outs", bufs=4) as outs,
            tc.tile_pool(name="ps_s", bufs=2, space="PSUM") as ps_s,
            tc.tile_pool(name="ps_kq", bufs=1, space="PSUM") as ps_kq,
            tc.tile_pool(name="ps_vpv", bufs=1, space="PSUM") as ps_vpv,
        ):
            # ---------- constants ----------
            w_sb = consts.tile([128, KCH, 192], bf16, tag="w")
            nc.sync.dma_start(out=w_sb, in_=w_ap)
            mask_sb = consts.tile([128, 128], bf16, tag="mask")
            nc.sync.dma_start(out=mask_sb, in_=mask_ap)
            # V with ones column: slot aa is [128, 65], col 64 preset to 1.0
            vt_sb = consts.tile([128, NTS, 65], bf16, tag="vt")
            nc.gpsimd.memset(vt_sb, 1.0)
            kq_sb = consts.tile([128, T], bf16, tag="kq")
            qk_sb = consts.tile([64, T], bf16, tag="qk")
            # warm the exp table so ACT_TABLE_LOAD doesn't hit the first unit
            warm = consts.tile([128, 1], bf16, tag="warm")
            nc.gpsimd.memset(warm, 0.0)
            nc.scalar.activation(warm, warm, EXP, scale=1.0)

            xt_t = {}

            def dma_x(j):
                xt_t[j] = xp.tile([128, KCH, 512], bf16, tag="x", name=f"xt{j}")
                nc.sync.dma_start(out=xt_t[j], in_=xt_ap[:, :, j * 512:(j + 1) * 512])

            kq_ps = {}
            v_ps = {}

            def emit_kq_half(j, half):
                """Half of KQ^T tile j (4 k-chunks); cast + Q-copy on half 1."""
                if half == 0:
                    kq_ps[j] = ps_kq.tile([128, 512], fp32, tag="kq", name=f"kq{j}")
                for k in range(4 * half, 4 * half + 4):
                    nc.tensor.matmul(
                        kq_ps[j],
                        w_sb[:, k, 0:128],
                        xt_t[j][:, k, :],
                        start=(k == 0),
                        stop=(k == KCH - 1),
                    )
                if half == 1:
                    nc.vector.tensor_copy(kq_sb[:, j * 512:(j + 1) * 512], kq_ps[j])
                    nc.sync.dma_start(
                        out=qk_sb[:, j * 512:(j + 1) * 512],
                        in_=kq_sb[64:128, j * 512:(j + 1) * 512],
                    )

            def emit_v_half(j, half):
                """Half of V tile j (4 k-chunks); copy-with-ones on half 1."""
                if half == 0:
                    v_ps[j] = ps_vpv.tile([128, 4, 64], fp32, tag="vpv", name=f"v{j}")
                for k in range(4 * half, 4 * half + 4):
                    for q in range(4):
                        nc.tensor.matmul(
                            v_ps[j][:, q, :],
                            xt_t[j][:, k, q * 128:(q + 1) * 128],
                            w_sb[:, k, 128:192],
                            start=(k == 0 and q == 0),
                            stop=(k == KCH - 1),
                            skip_group_check=True,
                        )
                if half == 1:
                    nc.vector.tensor_copy(vt_sb[:, 4 * j:4 * j + 4, 0:64], v_ps[j])

            pt = {}

            def emit_unit(j, unit):
                uw = sum(s[2] for s in unit)
                base = unit[0][3]
                psu = ps_s.tile([128, UNIT], fp32, tag="s")
                for (a, t0, w, off, start) in unit:
                    nc.tensor.matmul(
                        psu[:, off - base:off - base + w],
                        kq_sb[0:64, a * 128:(a + 1) * 128],
                        qk_sb[:, t0:t0 + w],
                        start=start,
                        stop=True,
                        skip_group_check=True,
                    )
                nc.scalar.activation(
                    pt[j][:, base:base + uw], psu[:, 0:uw], EXP, scale=0.125
                )

            def emit_masks(j, slots):
                # diagonal 128-block is the first 128 columns of each of the
                # four slots for chunks 4j..4j+3
                for (a, t0, w, off, start) in slots[-4:]:
                    nc.vector.tensor_mul(
                        pt[j][:, off:off + 128], pt[j][:, off:off + 128], mask_sb
                    )

            ogs = {}

            def emit_chain(i, pool):
                jj = i // 4
                smap = {s[0]: s for s in _tile_slots(jj)}
                po = pool.tile(
                    [128, 65], fp32,
                    tag="vpv" if pool is ps_vpv else "kq", name=f"po{i}",
                )
                for aa in range(i + 1):
                    (_, t0, _, off, _) = smap[aa]
                    col = off + 128 * i - t0
                    nc.tensor.matmul(
                        po,
                        pt[jj][:, col:col + 128],
                        vt_sb[:, aa, :],
                        start=(aa == 0),
                        stop=(aa == i),
                    )
                dr = outs.tile([128, 1], fp32, tag="dr")
                nc.vector.reciprocal(dr, po[:, 64:65])
                nc.vector.tensor_scalar_mul(ogs[jj][:, i % 4, :], po[:, 0:64], dr)

            def emit_filler(n):
                for _ in range(n):
                    nc.tensor.ldweights(kq_sb[0:64, 0:128])

            # ---------- pipeline ----------
            for j in range(3):
                dma_x(j)
            for h in range(2):
                emit_kq_half(0, h)
            for h in range(2):
                emit_v_half(0, h)

            for j in range(NTT):
                slots = _tile_slots(j)
                units = _units(slots)
                pt[j] = ptp.tile([128, PTW], bf16, tag="pt", name=f"pt{j}")

                # PE side-work interleaved between score units so the tensor
                # engine never idles (keeps the p-state ramped) and the scalar
                # engine is never blocked behind a stalled PE queue.
                work = []
                if j >= 1:
                    ogs[j - 1] = outs.tile(
                        [128, 4, 64], fp32, tag="og", name=f"og{j - 1}"
                    )
                    for i in range(4 * (j - 1), 4 * j):
                        work.append(lambda i=i: emit_chain(i, ps_vpv))
                    work.append(lambda j=j: nc.sync.dma_start(
                        out=o_ap[:, 4 * (j - 1):4 * j, :], in_=ogs[j - 1]
                    ))
                if j + 3 < NTT:
                    work.append(lambda j=j: dma_x(j + 3))
                if j + 1 < NTT:
                    for h in range(2):
                        work.append(lambda j=j, h=h: emit_kq_half(j + 1, h))
                    for h in range(2):
                        work.append(lambda j=j, h=h: emit_v_half(j + 1, h))

                done = 0
                for n, u in enumerate(units):
                    emit_unit(j, u)
                    hi = (n + 1) * len(work) // len(units)
                    while done < hi:
                        work[done]()
                        done += 1
                    emit_filler(FILL)
                emit_masks(j, slots)

            # tail: chains of tile 7, two banks in parallel (vpv + kq pools)
            ogs[NTT - 1] = outs.tile([128, 4, 64], fp32, tag="og", name="og7")
            for n, i in enumerate(range(4 * (NTT - 1), 4 * NTT)):
                emit_chain(i, ps_vpv if n % 2 == 0 else ps_kq)
            nc.sync.dma_start(out=o_ap[:, 4 * (NTT - 1):4 * NTT, :], in_=ogs[NTT - 1])

    nc.compile()
    return nc


def _get_nc():
    if "nc" not in _CACHE:
        _CACHE["nc"] = _build_program()
    return _CACHE["nc"]


def _prep_inputs(x, W):
    """Host-side packing shared by kernel() and test harnesses."""
    x = np.asarray(x, dtype=np.float32)
    W = np.asarray(W, dtype=np.float32)
    assert x.shape == (B, T, EMB) and W.shape == (EMB, 3 * HEAD)
    # [B, 128, KCH, T]: partition p of chunk k holds x[b, :, 128k+p]
    xt = np.ascontiguousarray(
        x.transpose(0, 2, 1).reshape(B, KCH, 128, T).transpose(0, 2, 1, 3)
    ).astype(BF16)
    w_r = np.ascontiguousarray(
        W.reshape(KCH, 128, 3 * HEAD)
    ).transpose(1, 0, 2).astype(BF16)
    w_r = np.ascontiguousarray(w_r)
    mask = np.triu(np.ones((128, 128), np.float32)).astype(BF16)
    return xt, w_r, mask


def kernel(x, W):
    from concourse.bass_utils import run_bass_kernel_spmd

    xt, w_r, mask = _prep_inputs(x, W)
    nc = _get_nc()
    in_maps = [{"xt": xt[b], "w": w_r, "mask": mask} for b in range(B)]
    res = run_bass_kernel_spmd(nc, in_maps, list(range(B)))
    # o[p, i, c] = out[128*i + p, c]
    return np.stack(
        [
            res.results[b]["o"].transpose(1, 0, 2).reshape(T, HEAD)
            for b in range(B)
        ]
    ).astype(np.float32)
